# revision 75
# baseline (speedup 1.0000x reference)
"""Graph-transformer attention block on 8 Trainium2 NeuronCores.

Reference math (N=8192, D=256, per-core q-shard QPC=1024):
    Q = h @ Wq.T; K = h @ Wk.T; V = h @ Wv.T
    S = (1/16) * (Q @ K.T) * adj          # multiplicative 0/1 mask
    A = softmax(S, axis=1)                # exp(0)=1 for non-edges!
    X = A @ V

Device algorithm (score tiles transposed: S_T[k, q]), using the 0/1-mask
identity  P = exp(S*adj) = 1 + adj*(exp(S)-1):
    S_T[k,q]  = K8 @ Q8^T                  (PE fp8 DoubleRow 3-pass residual,
                                            256-contraction per pass)
    tmp[k,q]  = (exp(S)-1) * adj/16        (ACT exp per q-half; mask stt DVE)
    U_T[e,q]  = sum_k tmp * 16*V[k,e]      (PE; hybrid precision by k-pair:
                                            even pairs fp8 DR vs a two-term
                                            fp8 residual of 16*V, odd pairs
                                            fp16 vs fp16 16*V)
    row[q]    = sum_k tmp[k,q]             (PE fp8 DR ones-reduce, inlined
                                            per pair into the main loop)
    X_T[e,q]  = (U_T + cs) / (N + 16*row)
Host prep: projections Q/K/V in fp32 (O(N*D^2), ~2% of total FLOPs),
quantized to the packed device layouts; adj cast to fp8 holding 1/16 per
edge; cs = colsum(V) exact.  The O(N^2*D) attention work runs on-device.

The hybrid A@V splits quantization noise: fp8 tmp carries ~0.9% relative
noise per edge weight; applying it to only half the k-range (with the V
side residual-corrected to ~0.01%) keeps total rel err ~1.5e-2 < 2e-2
while cutting the A@V PE time roughly in half vs all-fp16.
"""

import os
import sys

import numpy as np

for _p in ("/opt/trn_rl_repo", "/root/.axon_site/_ro/trn_rl_repo"):
    if os.path.isdir(_p) and _p not in sys.path:
        sys.path.insert(0, _p)

N = 8192
D = 256
NCORES = 8
QPC = N // NCORES  # 1024 query rows per core
P = 128
SCALE = 1.0 / 16.0
# A@V precision per 256-row k-pair: True -> fp8 DR (V residual-corrected),
# False -> fp16.  24/32 fp8 keeps rel err ~1.79e-2 < 2e-2 (Bresenham spread).
PAIR_KIND = [(tp * 3) % 4 < 3 for tp in range(32)]
PAIR8 = [tp for tp, k in enumerate(PAIR_KIND) if k]
PAIR16 = [tp for tp, k in enumerate(PAIR_KIND) if not k]
ORD8 = {tp: o for o, tp in enumerate(PAIR8)}
ORD16 = {tp: o for o, tp in enumerate(PAIR16)}
NF8 = len(PAIR8)
NF16 = len(PAIR16)

_CACHE = {}


def build_program(n_k=N, n_q=QPC):
    import concourse.bass as bass  # noqa: F401
    import concourse.tile as tile
    from concourse import bacc
    from concourse import mybir

    fp8 = mybir.dt.float8e4
    fp16 = mybir.dt.float16
    fp32 = mybir.dt.float32
    Alu = mybir.AluOpType
    Act = mybir.ActivationFunctionType
    DR = mybir.MatmulPerfMode.DoubleRow

    n_kt = n_k // P                 # 128-row k tiles
    n_pair = n_kt // 2              # 256-row pairs (A@V granularity)
    n_grp = n_pair // 2             # {fp8 pair, fp16 pair} groups
    qw = min(n_q, 512)
    n_qc = n_q // qw

    nc = bacc.Bacc(None)

    # q8d/k8d dim1 packs (residual r, contraction half dh) as r*2+dh
    q8d = nc.dram_tensor("q8d", [P, 4, n_q], fp8, kind="ExternalInput")
    k8d = nc.dram_tensor("k8d", [P, 4, n_k], fp8, kind="ExternalInput")
    # vab8d dim2 packs (residual term, pair-half jj) as res*2+jj
    vab8d = nc.dram_tensor("vab8d", [P, NF8, 4, D], fp8,
                           kind="ExternalInput")
    v16d = nc.dram_tensor("v16d", [P, NF16, 2, D], fp16, kind="ExternalInput")
    adjT = nc.dram_tensor("adjT", [n_k, n_q], fp8, kind="ExternalInput")
    csd = nc.dram_tensor("csd", [D, 1], fp32, kind="ExternalInput")
    xT = nc.dram_tensor("xT", [D, n_q], fp16, kind="ExternalOutput")

    with tile.TileContext(nc) as tc:
        with (
            tc.tile_pool(name="const", bufs=1) as cpool,
            tc.tile_pool(name="adjp", bufs=4) as adjp,
            tc.tile_pool(name="t16p", bufs=4) as t16p,
            tc.tile_pool(name="ep", bufs=6) as epool,
        ):
            # ---- inputs (few, consolidated DMAs) ----
            q8 = cpool.tile([P, 4, n_q], fp8, tag="q8")
            k8 = cpool.tile([P, 4, n_k], fp8, tag="k8")
            vab8 = cpool.tile([P, NF8, 4, D], fp8, tag="vab8")
            v16 = cpool.tile([P, NF16, 2, D], fp16, tag="v16")

            # adj quad tiles: one DMA covers 4 k-tiles (2 pairs)
            adj_tiles = {}

            def adj_mk(aq):
                adj_t = adjp.tile([P, 4, n_q], fp8, tag="adj", name=f"adj{aq}")
                adj_tiles[aq] = adj_t
                return adj_t[:]

            def adj_src(aq):
                return adjT[4 * aq * P:(4 * aq + 4) * P, :].rearrange(
                    "(j p) q -> p j q", p=P)

            def emit_adj(aq):
                if aq >= n_kt // 4 or aq in adj_tiles:
                    return
                nc.sync.dma_start(out=adj_mk(aq), in_=adj_src(aq))

            def kchunk(lo, hi):
                if lo < hi:
                    nc.sync.dma_start(out=k8[:, :, lo:hi],
                                      in_=k8d[:, :, lo:hi])

            def vabchunk(lo, hi):
                nc.sync.dma_start(out=vab8[:, lo:hi], in_=vab8d[:, lo:hi])

            def v16chunk(lo, hi):
                nc.sync.dma_start(out=v16[:, lo:hi], in_=v16d[:, lo:hi])

            # interleave loads on one queue in consumption order (the ~650ns
            # serial HWDGE issue cost per DMA makes ordering matter); later
            # adj quads HOL-block behind buffer frees, which conveniently
            # defers the late bulk chunks queued after them
            cs_all = cpool.tile([P, 2], fp32, tag="cs_all")
            kchunk(0, 512)
            nc.sync.dma_start(out=q8[:], in_=q8d[:, :, :])
            emit_adj(0)
            vabchunk(0, 4)
            kchunk(512, 1024)
            emit_adj(1)
            v16chunk(0, 2)
            kchunk(1024, 2048)
            emit_adj(2)
            kchunk(2048, 3072)
            vabchunk(4, 10)
            emit_adj(3)
            kchunk(3072, 4096)
            vabchunk(10, 17)
            v16chunk(2, 4)
            emit_adj(4)
            kchunk(4096, 5120)
            v16chunk(4, 6)
            kchunk(5120, 6144)
            vabchunk(17, NF8)
            kchunk(6144, n_k)
            emit_adj(5)
            v16chunk(6, NF16)
            nc.sync.dma_start(
                out=cs_all[:],
                in_=csd[:, :].rearrange("(e p) o -> p (e o)", p=P))

            def bulk_at(t):
                pass
            cs_sb = [cs_all[:, eh:eh + 1] for eh in range(2)]

            ones8 = cpool.tile([P, 2, 16], fp8, tag="ones8")
            nc.gpsimd.memset(ones8[:], 16.0)
            c32 = cpool.tile([P, 2, qw], fp8, tag="c32")
            nc.gpsimd.memset(c32[:], float(n_k) / (2 * P * 16.0))
            onec = cpool.tile([P, 1], fp32, tag="onec")
            nc.gpsimd.memset(onec[:], 1.0)
            row1 = cpool.tile([1, P], fp16, tag="row1")
            nc.gpsimd.memset(row1[:], 1.0)

            tmp8 = cpool.tile([P, n_kt, n_q], fp8, tag="tmp8")
            t16state = {}

            with tc.tile_pool(name="upsum", bufs=1, space="PSUM") as upsum:
                pu0 = upsum.tile([P, n_q], fp32, tag="pu0")
                pu1 = upsum.tile([P, n_q], fp32, tag="pu1")
                with tc.tile_pool(name="spsum", bufs=2, space="PSUM") as spsum:
                    pend = []  # delayed A@V emissions: (tp, t16 or None)
                    cnt8 = [0]

                    def emit_av(tp, t16, last):
                        is8_ = PAIR_KIND[tp]
                        g_ = ORD8[tp] if is8_ else ORD16[tp]
                        for qc in range(n_qc):
                            sl = slice(qc * qw, (qc + 1) * qw)
                            if is8_:
                                for eh, pu in ((0, pu0), (1, pu1)):
                                    for res in range(2):
                                        nc.tensor.matmul(
                                            pu[:, sl],
                                            vab8[:, g_, 2 * res:2 * res + 2,
                                                 eh * P:(eh + 1) * P],
                                            tmp8[:, 2 * tp:2 * tp + 2, sl],
                                            start=(tp == 0 and res == 0),
                                            stop=False,
                                            perf_mode=DR,
                                        )
                            else:
                                for jj in range(2):
                                    for eh, pu in ((0, pu0), (1, pu1)):
                                        nc.tensor.matmul(
                                            pu[:, sl],
                                            v16[:, g_, jj,
                                                eh * P:(eh + 1) * P],
                                            t16[:, jj, sl],
                                            start=False,
                                            stop=(last and jj == 1),
                                        )

                    def flush_av(last=False):
                        while pend:
                            tp_, t16_ = pend.pop(0)
                            emit_av(tp_, t16_, last)

                    for t in range(n_kt):
                        tp, j = t // 2, t % 2
                        is8 = PAIR_KIND[tp]
                        aq, aj = t // 4, t % 4
                        if aj == 0:
                            emit_adj(aq + 2)
                        adj_t = adj_tiles[aq]
                        # pending A@V from >=1 pair ago: emitted before this
                        # tile's S so a not-yet-ready mask can never block
                        # the PE queue head
                        if len(pend) > 2 or (pend and pend[0][0] < tp - 2):
                            emit_av(*pend.pop(0), last=False)
                        ps = spsum.tile([P, n_q], fp32, tag="ps", name=f"ps{t}")
                        if t == 0:
                            # p-state warmup: keep PE busy during the input
                            # DMA wait so the first real matmuls run at a
                            # higher clock (overwritten by S's start=True)
                            for _w in range(12):
                                nc.tensor.matmul(
                                    ps[0:1, 0:qw], ones8[:, :, 0:1],
                                    c32[:, :, 0:qw],
                                    start=True, stop=(_w == 11), perf_mode=DR,
                                )
                        for qc in range(n_qc):
                            for i, (rk, rq) in enumerate(
                                    ((0, 0), (0, 1), (1, 0))):
                                nc.tensor.matmul(
                                    ps[:, qc * qw:(qc + 1) * qw],
                                    k8[:, 2 * rk:2 * rk + 2,
                                       t * P:(t + 1) * P],
                                    q8[:, 2 * rq:2 * rq + 2,
                                       qc * qw:(qc + 1) * qw],
                                    start=(i == 0), stop=(i == 2),
                                    perf_mode=DR,
                                )
                        e_t = epool.tile([P, n_q], fp16, tag="e", name=f"e{t}")
                        nc.scalar.activation(e_t[:], ps[:], Act.Exp,
                                             scale=SCALE)
                        if is8:
                            # fp8 pair: tmp8 direct (DVE) feeds fp8 DR A@V
                            nc.vector.scalar_tensor_tensor(
                                tmp8[:, t, :], e_t[:], onec[:, 0:1],
                                adj_t[:, aj, :],
                                op0=Alu.subtract, op1=Alu.mult,
                            )
                        else:
                            # fp16 pair: t16 (DVE) feeds fp16 A@V; fp8 shadow
                            # copy (Pool, slack-tolerant) feeds the rowsum
                            if j == 0:
                                t16state["t"] = t16p.tile(
                                    [P, 2, n_q], fp16, tag="t16",
                                    name=f"t16_{tp}")
                            t16 = t16state["t"]
                            nc.vector.scalar_tensor_tensor(
                                t16[:, j, :], e_t[:], onec[:, 0:1],
                                adj_t[:, aj, :],
                                op0=Alu.subtract, op1=Alu.mult,
                            )
                            nc.gpsimd.tensor_copy(tmp8[:, t, :], t16[:, j, :])
                        if j == 1:
                            pend.append((tp, t16state.get("t")
                                         if not is8 else None))
                # ---- tail: rowsum + epilogue (in banks freed by ps) ----
                with tc.tile_pool(name="tpsum", bufs=1, space="PSUM") as tpsum:
                    # pr = N + 16*sum(tmp): ones8=16 scales, and one extra MM
                    # of constants adds N.  qc-pipelined: qc0's reciprocal/
                    # broadcast/epilogue overlap qc1's rowsum MMs.
                    pr = tpsum.tile([1, n_q], fp32, tag="pr")
                    rc32 = cpool.tile([1, n_q], fp32, tag="rc32")
                    rc = cpool.tile([1, n_q], fp16, tag="rc")
                    pb = tpsum.tile([P, n_q], fp32, tag="pb")
                    pb_sb = cpool.tile([P, n_q], fp32, tag="pb_sb")

                    def rowsum(qc):
                        sl = slice(qc * qw, (qc + 1) * qw)
                        nc.tensor.matmul(
                            pr[0:1, sl], ones8[:, :, 0:1], c32[:, :, 0:qw],
                            start=True, stop=False, perf_mode=DR,
                        )
                        for tp in range(n_pair):
                            nc.tensor.matmul(
                                pr[0:1, sl], ones8[:, :, 0:1],
                                tmp8[:, 2 * tp:2 * tp + 2, sl],
                                start=False, stop=(tp == n_pair - 1),
                                perf_mode=DR,
                            )

                    def epilogue(qc):
                        sl = slice(qc * qw, (qc + 1) * qw)
                        nc.vector.reciprocal_approx_fast(rc32[0:1, sl],
                                                         pr[0:1, sl])
                        with nc.allow_low_precision(
                                reason="1/rowsum fits fp16"):
                            nc.vector.tensor_copy(rc[0:1, sl], rc32[0:1, sl])
                        nc.tensor.matmul(
                            pb[:, sl], row1[:], rc[0:1, sl],
                            start=True, stop=True,
                        )
                        nc.scalar.activation(pb_sb[:, sl], pb[:, sl], Act.Copy)
                        for eh, pu in ((0, pu0), (1, pu1)):
                            x_sb = cpool.tile([P, n_q], fp16, tag=f"x{eh}",
                                              name=f"x{eh}_{qc}")
                            with nc.allow_low_precision(
                                    reason="fp16 out, rel tol 2e-2"):
                                nc.vector.scalar_tensor_tensor(
                                    x_sb[:, sl], pu[:, sl], cs_sb[eh][:, 0:1],
                                    pb_sb[:, sl],
                                    op0=Alu.add, op1=Alu.mult,
                                )
                            nc.sync.dma_start(
                                out=xT[eh * P:(eh + 1) * P, sl],
                                in_=x_sb[:, sl],
                            )

                    # rowsum qc0 first; the held-back last A@V pairs keep PE
                    # busy while qc0's reciprocal runs, then qc0's whole
                    # epilogue (pb MM included) slots in BEFORE rowsum qc1 so
                    # it drains on DVE/ACT/DMA underneath qc1's rowsum MMs
                    rowsum(0)
                    flush_av(last=True)
                    epilogue(0)
                    rowsum(1)
                    epilogue(1)

    nc.finalize()
    return nc


def _pack_kT(M8):
    """[n, 256] -> [128, 2, n]: out[p, dh, i] = M[i, dh*128+p]."""
    n = M8.shape[0]
    return np.ascontiguousarray(
        M8.T.reshape(2, P, n).transpose(1, 0, 2))


def _resid8(M):
    """Two-term fp8 residual decomposition of M (fp32)."""
    import ml_dtypes
    fp8 = ml_dtypes.float8_e4m3
    a = M.astype(fp8)
    b = (M - a.astype(np.float32)).astype(fp8)
    return a, b


def _host_prep(adj, h, Wq, Wk, Wv):
    import ml_dtypes

    fp8 = ml_dtypes.float8_e4m3
    h32 = h.astype(np.float32)
    Q = h32 @ Wq.T.astype(np.float32)
    K = h32 @ Wk.T.astype(np.float32)
    V32 = h32 @ Wv.T.astype(np.float32)
    # [128, 4, n]: dim1 = r*2+dh over the two fp8 residual terms
    q8_full = np.ascontiguousarray(np.stack(
        [_pack_kT(m) for m in _resid8(Q)]).transpose(
        1, 0, 2, 3).reshape(P, 4, N))
    k8_full = np.ascontiguousarray(np.stack(
        [_pack_kT(m) for m in _resid8(K)]).transpose(
        1, 0, 2, 3).reshape(P, 4, N))
    # V*16 tiled [128, n_pair, 2, 256]: vt[p, tp, jj, e] = 16*V[(2tp+jj)*128+p]
    vt = (16.0 * V32).reshape(N // P // 2, 2, P, D).transpose(2, 0, 1, 3)
    # fp8 pairs -> two-term residual packed [128, NF8, 4, 256] (res*2+jj);
    # fp16 pairs -> [128, NF16, 2, 256]
    v8sel = vt[:, PAIR8, :, :]                               # [128,NF8,2,256]
    va8_full = v8sel.astype(fp8)
    vb8_full = (v8sel - va8_full.astype(np.float32)).astype(fp8)
    vab8_full = np.ascontiguousarray(np.concatenate(
        [va8_full.reshape(P, NF8, 1, 2, D),
         vb8_full.reshape(P, NF8, 1, 2, D)], axis=2).reshape(P, NF8, 4, D))
    v16_full = np.ascontiguousarray(vt[:, PAIR16, :, :]).astype(np.float16)
    # adj encodes 1/16 per edge so tmp=(e-1)*adj/16 stays inside fp8 range
    adjT8 = np.where(adj.T != 0, np.float32(SCALE), np.float32(0)).astype(fp8)
    # colsum(V) exact (U accumulates at scale 1: tmp/16 times 16*V)
    cs = V32.sum(axis=0, dtype=np.float64).astype(np.float32).reshape(D, 1)
    in_maps = []
    for c in range(NCORES):
        in_maps.append({
            "q8d": np.ascontiguousarray(q8_full[:, :, c * QPC:(c + 1) * QPC]),
            "k8d": k8_full,
            "vab8d": vab8_full,
            "v16d": v16_full,
            "adjT": np.ascontiguousarray(adjT8[:, c * QPC:(c + 1) * QPC]),
            "csd": cs,
        })
    return in_maps


def kernel(adj, h, Wq, Wk, Wv, _trace=False):
    from concourse.bass_utils import run_bass_kernel_spmd

    if "nc" not in _CACHE:
        _CACHE["nc"] = build_program()
    nc = _CACHE["nc"]
    in_maps = _host_prep(adj, h, Wq, Wk, Wv)
    res = run_bass_kernel_spmd(nc, in_maps, list(range(NCORES)), trace=_trace)
    out = np.empty([N, D], np.float32)
    for c in range(NCORES):
        out[c * QPC:(c + 1) * QPC, :] = np.asarray(
            res.results[c]["xT"], np.float32
        ).T
    if _trace:
        return out, res
    return out
